# revision 5
# baseline (speedup 1.0000x reference)
"""Trainium2 Bass kernel for nn_Decoder (probtorch decoder joint log-prob).

Math (reference):
    Factors[s,f,v] = exp(-d2[s,f,v] * exp(-widths[s,f]))
        d2 = |R_v|^2 - 2 R_v.C_sf + |C_sf|^2
    Ymean[s,t,v]  = sum_f Weights[s,t,f] * Factors[s,f,v]
    lp[s] = priors(Weights, Centers, Widths)
          + sum_{t,v} [ -0.5*((data-Ymean)/Snoise)^2 - log(Snoise) - 0.5*log(2pi) ]

With Snoise == const sigma (true for the generated inputs), the data term
decomposes exactly:
    sum (data - Ymean)^2 = t1 - 2*t2[s] + t3[s]
      t1    = sum data^2                    (S-independent)
      t2[s] = <G_s, W_s>,  G_s[f,t] = sum_v Factors[s,f,v] * data[t,v]
      t3[s] = <W_s^T W_s, B_s>, B_s[f,f'] = sum_v F[s,f,v] F[s,f',v]
All the O(V)-sized work (exponent matmul, exp, G, B, t1) runs on the 8
NeuronCores with V sharded 7500/core; the remaining contractions are O(S*T*F)
and run on host.

Device kernel (per core, V-shard padded to 7680 = 60 chunks of 128):
  - exponent e[v,sf] via one K=5 matmul: lhsT rows [x,y,z,|r|^2,1] (from R),
    rhs rows [2*invw*Cx, 2*invw*Cy, 2*invw*Cz, -invw, -invw*|C|^2]
  - Factors = ACT Exp(psum) -> SBUF  [128, 2*512]
  - G += dataT_half^T @ F      (psum accumulate over chunks)
  - T += dataT_half^T @ dataT_half   (diag gives t1)
  - B += F_pair^T @ F_pair     (diag 50x50 blocks give B_s)
Outputs per core: G [2,100,500], B-pairs [100,500], T [100,200].
"""

import os
import sys

for _p in ("/opt/trn_rl_repo",):
    if os.path.isdir(_p) and _p not in sys.path:
        sys.path.insert(0, _p)

import numpy as np

S, T, F, V = 10, 200, 50, 60000
NCORES = 8
VS = V // NCORES        # 7500 voxels per core
CHUNK = 128
NCH = 60                # chunks per core -> padded shard of 7680
VP = CHUNK * NCH
NPAIR = NCH // 2
SF = S * F              # 500
SFP = 512               # padded sf (psum bank = 512 fp32)
TH = T // 2             # 100
NBPAIR = S // 2         # 5 s-pairs for the Gram blocks
LOG_2PI = float(np.log(2.0 * np.pi))

LAST_EXEC_NS = None
LAST_RESULT = None
_CACHE = {}


def _build_nc():
    import concourse.tile as tile
    from concourse import bacc, mybir

    nc = bacc.Bacc("TRN2", target_bir_lowering=False)
    lhsT5 = nc.dram_tensor("lhst5", [5, VP], mybir.dt.float32, kind="ExternalInput")
    rhs5 = nc.dram_tensor("rhs5", [5, SFP], mybir.dt.float32, kind="ExternalInput")
    dataT = nc.dram_tensor("datat", [VP, T], mybir.dt.float32, kind="ExternalInput")
    g_out = nc.dram_tensor("g_out", [2, TH, SF], mybir.dt.float32, kind="ExternalOutput")
    b_out = nc.dram_tensor("b_out", [TH, SF], mybir.dt.float32, kind="ExternalOutput")
    t_out = nc.dram_tensor("t_out", [TH, T], mybir.dt.float32, kind="ExternalOutput")

    Exp = mybir.ActivationFunctionType.Exp

    with tile.TileContext(nc) as tc:
        with (
            tc.tile_pool(name="consts", bufs=1) as consts,
            tc.tile_pool(name="dpool", bufs=4) as dpool,
            tc.tile_pool(name="fpool", bufs=2) as fpool,
            tc.tile_pool(name="opool", bufs=1) as opool,
            tc.tile_pool(name="pe_pool", bufs=2, space="PSUM") as pe_pool,
            tc.tile_pool(name="pacc", bufs=1, space="PSUM") as pacc,
        ):
            lhsT5_sb = consts.tile([5, VP], mybir.dt.float32)
            nc.sync.dma_start(out=lhsT5_sb, in_=lhsT5[:, :])
            rhs5_sb = consts.tile([5, SFP], mybir.dt.float32)
            nc.sync.dma_start(out=rhs5_sb, in_=rhs5[:, :])

            # Persistent psum accumulators (banks: G=2, B=1, T=1)
            pG = pacc.tile([128, 2 * SFP], mybir.dt.float32)
            pB = pacc.tile([128, SFP], mybir.dt.float32)
            pT = pacc.tile([128, SFP], mybir.dt.float32)

            def emit_exponent(j):
                """d2 matmuls for chunk pair j -> psum [128, 2*SFP]."""
                pE = pe_pool.tile([128, 2 * SFP], mybir.dt.float32, name="pE", tag="pE")
                dts = []
                for c in range(2):
                    ch = 2 * j + c
                    dt_t = dpool.tile([128, T], mybir.dt.float32, name="dt", tag="dt")
                    nc.sync.dma_start(
                        out=dt_t, in_=dataT[ch * CHUNK:(ch + 1) * CHUNK, :]
                    )
                    dts.append(dt_t)
                    nc.tensor.matmul(
                        out=pE[:, c * SFP:(c + 1) * SFP],
                        lhsT=lhsT5_sb[:, ch * CHUNK:(ch + 1) * CHUNK],
                        rhs=rhs5_sb,
                        start=True,
                        stop=True,
                    )
                return pE, dts

            def emit_exp(pE):
                f_sb = fpool.tile([128, 2 * SFP], mybir.dt.float32, name="f_sb", tag="f")
                nc.scalar.activation(out=f_sb, in_=pE, func=Exp)
                return f_sb

            def emit_accum(j, f_sb, dts):
                for c in range(2):
                    ch = 2 * j + c
                    first = ch == 0
                    last = ch == NCH - 1
                    fc = f_sb[:, c * SFP: c * SFP + SF]
                    dt_t = dts[c]
                    for th in range(2):
                        w = dt_t[:, th * TH:(th + 1) * TH]
                        # G: one bank per t-half
                        nc.tensor.matmul(
                            out=pG[0:TH, th * SFP: th * SFP + SF],
                            lhsT=w,
                            rhs=fc,
                            start=first,
                            stop=last,
                        )
                        # T: both halves share one bank
                        nc.tensor.matmul(
                            out=pT[0:TH, th * TH:(th + 1) * TH],
                            lhsT=w,
                            rhs=w,
                            start=first and th == 0,
                            stop=last and th == 1,
                        )
                    for p in range(NBPAIR):
                        fp_ = fc[:, p * TH:(p + 1) * TH]
                        nc.tensor.matmul(
                            out=pB[0:TH, p * TH:(p + 1) * TH],
                            lhsT=fp_,
                            rhs=fp_,
                            start=first and p == 0,
                            stop=last and p == NBPAIR - 1,
                        )

            # Software pipeline: issue next pair's exponent matmuls before this
            # pair's accumulation matmuls so PE never stalls on ACT.
            pE_cur, dts_cur = emit_exponent(0)
            for j in range(NPAIR):
                f_sb = emit_exp(pE_cur)
                if j + 1 < NPAIR:
                    pE_nxt, dts_nxt = emit_exponent(j + 1)
                emit_accum(j, f_sb, dts_cur)
                if j + 1 < NPAIR:
                    pE_cur, dts_cur = pE_nxt, dts_nxt

            g_copy = opool.tile([128, 2 * SFP], mybir.dt.float32)
            for th in range(2):
                nc.vector.tensor_copy(
                    out=g_copy[0:TH, th * SFP: th * SFP + SF],
                    in_=pG[0:TH, th * SFP: th * SFP + SF],
                )
            b_copy = opool.tile([128, SF], mybir.dt.float32)
            nc.vector.tensor_copy(out=b_copy[0:TH, :], in_=pB[0:TH, 0:SF])
            t_copy = opool.tile([128, T], mybir.dt.float32)
            nc.vector.tensor_copy(out=t_copy[0:TH, :], in_=pT[0:TH, 0:T])
            for th in range(2):
                nc.sync.dma_start(
                    out=g_out[th, :, :],
                    in_=g_copy[0:TH, th * SFP: th * SFP + SF],
                )
            nc.sync.dma_start(out=b_out[:, :], in_=b_copy[0:TH, :])
            nc.sync.dma_start(out=t_out[:, :], in_=t_copy[0:TH, :])

    nc.compile()
    return nc


def _host_prep(data, R, FactorCenters, FactorWidths):
    """Per-core DRAM inputs: lhsT5 [5,VP], dataT [VP,T] per core; rhs5 [5,SFP]."""
    R64 = np.asarray(R, np.float64)           # [V, 3]
    C64 = np.asarray(FactorCenters, np.float64).reshape(SF, 3)  # [sf, 3]
    w64 = np.asarray(FactorWidths, np.float64).reshape(SF)
    invw = np.exp(-w64)                        # [sf]
    c2 = np.sum(C64 * C64, axis=1)             # [sf]

    rhs5 = np.zeros((5, SFP), np.float32)
    rhs5[0:3, :SF] = (2.0 * invw * C64.T).astype(np.float32)
    rhs5[3, :SF] = (-invw).astype(np.float32)
    rhs5[4, :SF] = (-invw * c2).astype(np.float32)

    lhsT5_list = []
    dataT_list = []
    for c in range(NCORES):
        sl = slice(c * VS, (c + 1) * VS)
        Rc = R64[sl]                           # [VS, 3]
        lhsT5 = np.zeros((5, VP), np.float32)
        lhsT5[0:3, :VS] = Rc.T.astype(np.float32)
        lhsT5[3, :VS] = np.sum(Rc * Rc, axis=1).astype(np.float32)
        lhsT5[3, VS:] = 1.0e30                 # padding: exponent -> -inf -> F=0
        lhsT5[4, :] = 1.0
        lhsT5_list.append(lhsT5)

        dT = np.zeros((VP, T), np.float32)
        dT[:VS, :] = np.ascontiguousarray(np.asarray(data, np.float32)[:, sl].T)
        dataT_list.append(dT)
    return rhs5, lhsT5_list, dataT_list


def _run_device(rhs5, lhsT5_list, dataT_list, trace=False):
    global LAST_EXEC_NS, LAST_RESULT
    from concourse.bass_utils import run_bass_kernel_spmd

    if "nc" not in _CACHE:
        _CACHE["nc"] = _build_nc()
    nc = _CACHE["nc"]
    in_maps = [
        {"lhst5": lhsT5_list[c], "rhs5": rhs5, "datat": dataT_list[c]}
        for c in range(NCORES)
    ]
    res = run_bass_kernel_spmd(
        nc, in_maps, core_ids=list(range(NCORES)), trace=trace
    )
    LAST_EXEC_NS = res.exec_time_ns
    LAST_RESULT = res
    return res.results


def _make_sharded_runner(nc, in_maps):
    """Device-resident repeat-runner mirroring run_bass_via_pjrt's multi-core
    path, for timing NEFF executions without re-transferring inputs."""
    import jax
    from jax.experimental.shard_map import shard_map
    from jax.sharding import Mesh, NamedSharding, PartitionSpec
    from concourse import mybir
    from concourse.bass2jax import (
        _bass_exec_p,
        install_neuronx_cc_hook,
        partition_id_tensor,
    )

    install_neuronx_cc_hook()
    partition_name = nc.partition_id_tensor.name if nc.partition_id_tensor else None
    in_names, out_names, out_avals, zero_outs = [], [], [], []
    for alloc in nc.m.functions[0].allocations:
        if not isinstance(alloc, mybir.MemoryLocationSet):
            continue
        name = alloc.memorylocations[0].name
        if alloc.kind == "ExternalInput":
            if name != partition_name:
                in_names.append(name)
        elif alloc.kind == "ExternalOutput":
            out_names.append(name)
            shape = tuple(alloc.tensor_shape)
            dtype = mybir.dt.np(alloc.dtype)
            out_avals.append(jax.core.ShapedArray(shape, dtype))
            zero_outs.append(np.zeros(shape, dtype))
    n_params = len(in_names)
    n_outs = len(out_avals)
    all_in_names = list(in_names) + list(out_names)
    if partition_name is not None:
        all_in_names.append(partition_name)

    def _body(*args):
        operands = list(args)
        if partition_name is not None:
            operands.append(partition_id_tensor())
        outs = _bass_exec_p.bind(
            *operands,
            out_avals=tuple(out_avals),
            in_names=tuple(all_in_names),
            out_names=tuple(out_names),
            lowering_input_output_aliases=(),
            sim_require_finite=True,
            sim_require_nnan=True,
            nc=nc,
        )
        return tuple(outs)

    n = len(in_maps)
    devices = jax.devices()[:n]
    mesh = Mesh(np.asarray(devices), ("core",))
    in_specs = (PartitionSpec("core"),) * (n_params + n_outs)
    out_specs = (PartitionSpec("core"),) * n_outs
    donate = tuple(range(n_params, n_params + n_outs))
    sharded = jax.jit(
        shard_map(_body, mesh=mesh, in_specs=in_specs, out_specs=out_specs,
                  check_rep=False),
        donate_argnums=donate,
        keep_unused=True,
    )
    sh = NamedSharding(mesh, PartitionSpec("core"))
    dev_in = [
        jax.device_put(
            np.concatenate([np.asarray(in_maps[c][nm]) for c in range(n)], axis=0), sh
        )
        for nm in in_names
    ]

    def run_once():
        zeros = [
            jax.device_put(np.zeros((n * z.shape[0], *z.shape[1:]), z.dtype), sh)
            for z in zero_outs
        ]
        outs = sharded(*dev_in, *zeros)
        jax.block_until_ready(outs)
        return outs

    return run_once, out_names, out_avals


def bench_device(rhs5, lhsT5_list, dataT_list, iters=20, warmup=3):
    """Return (per_call_seconds_list, floor_seconds_list) using a trivial NEFF
    to estimate the axon dispatch floor."""
    import time

    if "nc" not in _CACHE:
        _CACHE["nc"] = _build_nc()
    in_maps = [
        {"lhst5": lhsT5_list[c], "rhs5": rhs5, "datat": dataT_list[c]}
        for c in range(NCORES)
    ]
    run_once, _, _ = _make_sharded_runner(_CACHE["nc"], in_maps)
    for _ in range(warmup):
        run_once()
    times = []
    for _ in range(iters):
        t0 = time.perf_counter()
        run_once()
        times.append(time.perf_counter() - t0)

    # trivial kernel: single small DMA through SBUF
    if "nc_floor" not in _CACHE:
        _CACHE["nc_floor"] = _build_floor_nc()
    floor_in = [{"x_in": np.ones((128, 16), np.float32)} for _ in range(NCORES)]
    run_floor, _, _ = _make_sharded_runner(_CACHE["nc_floor"], floor_in)
    for _ in range(warmup):
        run_floor()
    floors = []
    for _ in range(iters):
        t0 = time.perf_counter()
        run_floor()
        floors.append(time.perf_counter() - t0)
    return times, floors


def _build_floor_nc():
    import concourse.tile as tile
    from concourse import bacc, mybir

    nc = bacc.Bacc("TRN2", target_bir_lowering=False)
    x_in = nc.dram_tensor("x_in", [128, 16], mybir.dt.float32, kind="ExternalInput")
    y_out = nc.dram_tensor("y_out", [128, 16], mybir.dt.float32, kind="ExternalOutput")
    with tile.TileContext(nc) as tc:
        with tc.tile_pool(name="p", bufs=1) as pool:
            t = pool.tile([128, 16], mybir.dt.float32)
            nc.sync.dma_start(out=t, in_=x_in[:, :])
            nc.sync.dma_start(out=y_out[:, :], in_=t)
    nc.compile()
    return nc


def _normal_lp_sum(x, mu, sigma, axes):
    x = np.asarray(x, np.float64)
    mu = np.asarray(mu, np.float64)
    sigma = np.asarray(sigma, np.float64)
    z = (x - mu) / sigma
    lp = -0.5 * z * z - np.log(sigma) - 0.5 * LOG_2PI
    return np.sum(lp, axis=axes)


def _reference_fallback(data, R, Weights, FactorCenters, FactorWidths,
                        MeanWeight, SigmaWeight, MeanFactorCenter,
                        SigmaFactorCenter, MeanFactorWidth, SigmaFactorWidth,
                        Snoise):
    """Pure numpy path for inputs outside the expected regime (non-constant
    Snoise). Correct for arbitrary inputs, not performance-tuned."""
    R64 = np.asarray(R, np.float64)
    C64 = np.asarray(FactorCenters, np.float64)
    w64 = np.asarray(FactorWidths, np.float64)
    lp = _normal_lp_sum(Weights, MeanWeight[None], SigmaWeight[None], (1, 2))
    lp = lp + _normal_lp_sum(FactorCenters, MeanFactorCenter[None],
                             SigmaFactorCenter[None], (1, 2))
    lp = lp + _normal_lp_sum(FactorWidths, MeanFactorWidth[None],
                             SigmaFactorWidth[None], (1,))
    data64 = np.asarray(data, np.float64)
    Sn64 = np.asarray(Snoise, np.float64)
    W64 = np.asarray(Weights, np.float64)
    r2 = np.sum(R64 * R64, axis=-1)
    c2 = np.sum(C64 * C64, axis=-1)
    CHV = 4096
    acc = np.zeros(S, np.float64)
    log_term = -np.sum(np.log(Sn64)) - 0.5 * LOG_2PI * T * V
    for v0 in range(0, V, CHV):
        v1 = min(v0 + CHV, V)
        cross = np.einsum("sfk,vk->sfv", C64, R64[v0:v1])
        d2 = r2[None, None, v0:v1] - 2.0 * cross + c2[..., None]
        Fa = np.exp(-d2 * np.exp(-w64)[..., None])
        Ym = np.einsum("stf,sfv->stv", W64, Fa)
        z = (data64[None, :, v0:v1] - Ym) / Sn64[None, :, v0:v1]
        acc += -0.5 * np.sum(z * z, axis=(1, 2))
    return (lp + acc + log_term).astype(np.float32)


def kernel(data, R, Weights, FactorCenters, FactorWidths,
           MeanWeight, SigmaWeight, MeanFactorCenter, SigmaFactorCenter,
           MeanFactorWidth, SigmaFactorWidth, Snoise, _trace=False):
    Snoise = np.asarray(Snoise)
    smin, smax = float(Snoise.min()), float(Snoise.max())
    if smin != smax or smin <= 0.0:
        return _reference_fallback(
            data, R, Weights, FactorCenters, FactorWidths, MeanWeight,
            SigmaWeight, MeanFactorCenter, SigmaFactorCenter, MeanFactorWidth,
            SigmaFactorWidth, Snoise)
    sigma = smin

    rhs5, lhsT5_list, dataT_list = _host_prep(data, R, FactorCenters, FactorWidths)
    results = _run_device(rhs5, lhsT5_list, dataT_list, trace=_trace)

    # Gather + final contractions (all O(S*T*F), fp64 on host)
    Gsum = np.zeros((T, SF), np.float64)
    Bsum = np.zeros((TH, SF), np.float64)
    Tsum = np.zeros((TH, T), np.float64)
    for r in results:
        g = np.asarray(r["g_out"], np.float64)     # [2, TH, SF]
        Gsum[:TH] += g[0]
        Gsum[TH:] += g[1]
        Bsum += np.asarray(r["b_out"], np.float64)
        Tsum += np.asarray(r["t_out"], np.float64)

    W64 = np.asarray(Weights, np.float64)          # [S, T, F]
    # t2[s] = sum_{t,f} W[s,t,f] * G[t, s*F+f]
    G3 = Gsum.reshape(T, S, F)
    t2 = np.einsum("stf,tsf->s", W64, G3)
    # B_s blocks from pair layout
    t3 = np.zeros(S, np.float64)
    for p in range(NBPAIR):
        B0 = Bsum[0:F, p * TH: p * TH + F]
        B1 = Bsum[F:2 * F, p * TH + F: p * TH + 2 * F]
        D0 = W64[2 * p].T @ W64[2 * p]
        D1 = W64[2 * p + 1].T @ W64[2 * p + 1]
        t3[2 * p] = np.sum(D0 * B0)
        t3[2 * p + 1] = np.sum(D1 * B1)
    idx = np.arange(TH)
    t1 = float(Tsum[idx, idx].sum() + Tsum[idx, TH + idx].sum())

    z2sum = (t1 - 2.0 * t2 + t3) / (sigma * sigma)
    lp_data = -0.5 * z2sum - T * V * (np.log(sigma) + 0.5 * LOG_2PI)

    lp = _normal_lp_sum(Weights, np.asarray(MeanWeight)[None],
                        np.asarray(SigmaWeight)[None], (1, 2))
    lp = lp + _normal_lp_sum(FactorCenters, np.asarray(MeanFactorCenter)[None],
                             np.asarray(SigmaFactorCenter)[None], (1, 2))
    lp = lp + _normal_lp_sum(FactorWidths, np.asarray(MeanFactorWidth)[None],
                             np.asarray(SigmaFactorWidth)[None], (1,))
    return (lp + lp_data).astype(np.float32)


# revision 12
# speedup vs baseline: 1.5772x; 1.5772x over previous
"""Trainium2 Bass kernel for nn_Decoder (probtorch decoder joint log-prob).

Math (reference):
    Factors[s,f,v] = exp(-d2[s,f,v] * exp(-widths[s,f]))
        d2 = |R_v|^2 - 2 R_v.C_sf + |C_sf|^2
    Ymean[s,t,v]  = sum_f Weights[s,t,f] * Factors[s,f,v]
    lp[s] = priors(Weights, Centers, Widths)
          + sum_{t,v} [ -0.5*((data-Ymean)/Snoise)^2 - log(Snoise) - 0.5*log(2pi) ]

With Snoise == const sigma (true for the generated inputs), the data term
decomposes exactly:
    sum (data - Ymean)^2 = t1 - 2*t2[s] + t3[s]
      t1    = sum data^2                    (S-independent)
      t2[s] = <G_s, W_s>,  G_s[f,t] = sum_v Factors[s,f,v] * data[t,v]
      t3[s] = <W_s^T W_s, B_s>, B_s[f,f'] = sum_v F[s,f,v] F[s,f',v]
All the O(V)-sized work (exponent matmul, exp, G, B, t1) runs on the 8
NeuronCores with V sharded 7500/core; the remaining contractions are O(S*T*F)
and run on host.

Device kernel (per core, V-shard padded to 7680 = 60 chunks of 128):
  - exponent e[v,sf] via one K=5 matmul: lhsT rows [x,y,z,|r|^2,1] (from R),
    rhs rows [2*invw*Cx, 2*invw*Cy, 2*invw*Cz, -invw, -invw*|C|^2]
  - Factors = ACT Exp(psum) -> SBUF  [128, 2*512]
  - G += dataT_half^T @ F      (psum accumulate over chunks)
  - T += dataT_half^T @ dataT_half   (diag gives t1)
  - B += F_pair^T @ F_pair     (diag 50x50 blocks give B_s)
Outputs per core: G [2,100,500], B-pairs [100,500], T [100,200].
"""

import os
import sys

for _p in ("/opt/trn_rl_repo",):
    if os.path.isdir(_p) and _p not in sys.path:
        sys.path.insert(0, _p)

import numpy as np

S, T, F, V = 10, 200, 50, 60000
NCORES = 8
VS = V // NCORES        # 7500 voxels per core
CHUNK = 128
NCH = 60                # chunks per core -> padded shard of 7680
VP = CHUNK * NCH
NPAIR = NCH // 2
SF = S * F              # 500
SFP = 512               # padded sf (psum bank = 512 fp32)
TH = T // 2             # 100
NBPAIR = S // 2         # 5 s-pairs for the Gram blocks
KE = 14                 # exponent-matmul contraction (hi/lo bf16 split)
LOG_2PI = float(np.log(2.0 * np.pi))

LAST_EXEC_NS = None
LAST_RESULT = None
_CACHE = {}


def _build_nc():
    import concourse.tile as tile
    from concourse import bacc, mybir

    nc = bacc.Bacc("TRN2", target_bir_lowering=False)
    lhsT5 = nc.dram_tensor("lhst5", [KE, VP], mybir.dt.bfloat16, kind="ExternalInput")
    rhs5 = nc.dram_tensor("rhs5", [KE, SFP], mybir.dt.bfloat16, kind="ExternalInput")
    dataT = nc.dram_tensor("datat", [VP, T], mybir.dt.bfloat16, kind="ExternalInput")
    g_out = nc.dram_tensor("g_out", [2, TH, SF], mybir.dt.float32, kind="ExternalOutput")
    b_out = nc.dram_tensor("b_out", [TH, SF], mybir.dt.float32, kind="ExternalOutput")
    t_out = nc.dram_tensor("t_out", [TH, T], mybir.dt.float32, kind="ExternalOutput")

    Exp = mybir.ActivationFunctionType.Exp

    with tile.TileContext(nc) as tc:
        with (
            tc.tile_pool(name="consts", bufs=1) as consts,
            tc.tile_pool(name="dpool", bufs=4) as dpool,
            tc.tile_pool(name="fpool", bufs=2) as fpool,
            tc.tile_pool(name="opool", bufs=1) as opool,
            tc.tile_pool(name="pe_pool", bufs=2, space="PSUM") as pe_pool,
            tc.tile_pool(name="pacc", bufs=1, space="PSUM") as pacc,
        ):
            lhsT5_sb = consts.tile([KE, VP], mybir.dt.bfloat16)
            nc.sync.dma_start(out=lhsT5_sb, in_=lhsT5[:, :])
            rhs5_sb = consts.tile([KE, SFP], mybir.dt.bfloat16)
            nc.sync.dma_start(out=rhs5_sb, in_=rhs5[:, :])

            # Persistent psum accumulators (banks: G=2, B=1, T=1)
            pG = pacc.tile([128, 2 * SFP], mybir.dt.float32)
            pB = pacc.tile([128, SFP], mybir.dt.float32)
            pT = pacc.tile([128, SFP], mybir.dt.float32)

            def emit_exponent(j):
                """d2 matmuls for chunk pair j -> psum [128, 2*SFP]."""
                pE = pe_pool.tile([128, 2 * SFP], mybir.dt.float32, name="pE", tag="pE")
                dts = []
                for c in range(2):
                    ch = 2 * j + c
                    dt_t = dpool.tile([128, T], mybir.dt.bfloat16, name="dt", tag="dt")
                    nc.sync.dma_start(
                        out=dt_t, in_=dataT[ch * CHUNK:(ch + 1) * CHUNK, :]
                    )
                    dts.append(dt_t)
                    nc.tensor.matmul(
                        out=pE[:, c * SFP:(c + 1) * SFP],
                        lhsT=lhsT5_sb[:, ch * CHUNK:(ch + 1) * CHUNK],
                        rhs=rhs5_sb,
                        start=True,
                        stop=True,
                    )
                return pE, dts

            def emit_exp(pE):
                f_sb = fpool.tile([128, 2 * SFP], mybir.dt.bfloat16, name="f_sb", tag="f")
                nc.scalar.activation(out=f_sb, in_=pE, func=Exp)
                return f_sb

            def emit_accum(j, f_sb, dts):
                for c in range(2):
                    ch = 2 * j + c
                    first = ch == 0
                    last = ch == NCH - 1
                    fc = f_sb[:, c * SFP: c * SFP + SF]
                    dt_t = dts[c]
                    for th in range(2):
                        w = dt_t[:, th * TH:(th + 1) * TH]
                        # G: one bank per t-half
                        nc.tensor.matmul(
                            out=pG[0:TH, th * SFP: th * SFP + SF],
                            lhsT=w,
                            rhs=fc,
                            start=first,
                            stop=last,
                        )
                        # T: both halves share one bank
                        nc.tensor.matmul(
                            out=pT[0:TH, th * TH:(th + 1) * TH],
                            lhsT=w,
                            rhs=w,
                            start=first and th == 0,
                            stop=last and th == 1,
                        )
                    for p in range(NBPAIR):
                        fp_ = fc[:, p * TH:(p + 1) * TH]
                        nc.tensor.matmul(
                            out=pB[0:TH, p * TH:(p + 1) * TH],
                            lhsT=fp_,
                            rhs=fp_,
                            start=first and p == 0,
                            stop=last and p == NBPAIR - 1,
                        )

            # Software pipeline: issue next pair's exponent matmuls before this
            # pair's accumulation matmuls so PE never stalls on ACT.
            pE_cur, dts_cur = emit_exponent(0)
            for j in range(NPAIR):
                f_sb = emit_exp(pE_cur)
                if j + 1 < NPAIR:
                    pE_nxt, dts_nxt = emit_exponent(j + 1)
                emit_accum(j, f_sb, dts_cur)
                if j + 1 < NPAIR:
                    pE_cur, dts_cur = pE_nxt, dts_nxt

            g_copy = opool.tile([128, 2 * SFP], mybir.dt.float32)
            for th in range(2):
                nc.vector.tensor_copy(
                    out=g_copy[0:TH, th * SFP: th * SFP + SF],
                    in_=pG[0:TH, th * SFP: th * SFP + SF],
                )
            b_copy = opool.tile([128, SF], mybir.dt.float32)
            nc.vector.tensor_copy(out=b_copy[0:TH, :], in_=pB[0:TH, 0:SF])
            t_copy = opool.tile([128, T], mybir.dt.float32)
            nc.vector.tensor_copy(out=t_copy[0:TH, :], in_=pT[0:TH, 0:T])
            for th in range(2):
                nc.sync.dma_start(
                    out=g_out[th, :, :],
                    in_=g_copy[0:TH, th * SFP: th * SFP + SF],
                )
            nc.sync.dma_start(out=b_out[:, :], in_=b_copy[0:TH, :])
            nc.sync.dma_start(out=t_out[:, :], in_=t_copy[0:TH, :])

    nc.compile()
    return nc


def _host_prep(data, R, FactorCenters, FactorWidths):
    """Per-core DRAM inputs: lhsT [KE,VP] bf16, dataT [VP,T] bf16 per core;
    rhs [KE,SFP] bf16 shared.

    The exponent e = 2*invw*(R.C) - invw*|R|^2 - invw*|C|^2 is computed by a
    K=KE bf16 matmul using hi/lo splitting for fp32-grade accuracy:
    each product L*M becomes Lh*Mh + Lh*Ml + Ll*Mh (3 rows)."""
    import ml_dtypes

    bf16 = ml_dtypes.bfloat16
    R64 = np.asarray(R, np.float64)           # [V, 3]
    C64 = np.asarray(FactorCenters, np.float64).reshape(SF, 3)  # [sf, 3]
    w64 = np.asarray(FactorWidths, np.float64).reshape(SF)
    invw = np.exp(-w64)                        # [sf]
    c2 = np.sum(C64 * C64, axis=1)             # [sf]

    def split(a):
        h = a.astype(bf16).astype(np.float64)
        l = (a - h).astype(bf16).astype(np.float64)
        return h, l

    m_terms = [2.0 * invw * C64[:, 0], 2.0 * invw * C64[:, 1],
               2.0 * invw * C64[:, 2], -invw]
    rhs_rows = []
    for M in m_terms:
        Mh, Ml = split(M)
        rhs_rows += [Mh, Ml, Mh]
    m4h, m4l = split(-invw * c2)
    rhs_rows += [m4h, m4l]
    rhs = np.zeros((KE, SFP), bf16)
    rhs[:, :SF] = np.stack(rhs_rows).astype(bf16)

    data32 = np.asarray(data, np.float32)
    lhsT_list = []
    dataT_list = []
    for c in range(NCORES):
        sl = slice(c * VS, (c + 1) * VS)
        Rc = R64[sl]                           # [VS, 3]
        l_terms = [Rc[:, 0], Rc[:, 1], Rc[:, 2], np.sum(Rc * Rc, axis=1)]
        rows = []
        for L in l_terms:
            Lh, Ll = split(L)
            rows += [Lh, Lh, Ll]
        rows += [np.ones(VS), np.ones(VS)]
        lhsT = np.zeros((KE, VP), bf16)
        lhsT[:, :VS] = np.stack(rows).astype(bf16)
        lhsT[9, VS:] = bf16(1.0e30)            # r2h row: padding -> exp(-huge)=0
        lhsT[12, VS:] = bf16(1.0)
        lhsT[13, VS:] = bf16(1.0)
        lhsT_list.append(lhsT)

        dT = np.zeros((VP, T), bf16)
        dT[:VS, :] = np.ascontiguousarray(data32[:, sl].T).astype(bf16)
        dataT_list.append(dT)
    return rhs, lhsT_list, dataT_list


def _run_device(rhs5, lhsT5_list, dataT_list, trace=False):
    global LAST_EXEC_NS, LAST_RESULT
    from concourse.bass_utils import run_bass_kernel_spmd

    if "nc" not in _CACHE:
        _CACHE["nc"] = _build_nc()
    nc = _CACHE["nc"]
    in_maps = [
        {"lhst5": lhsT5_list[c], "rhs5": rhs5, "datat": dataT_list[c]}
        for c in range(NCORES)
    ]
    res = run_bass_kernel_spmd(
        nc, in_maps, core_ids=list(range(NCORES)), trace=trace
    )
    LAST_EXEC_NS = res.exec_time_ns
    LAST_RESULT = res
    return res.results


def _make_sharded_runner(nc, in_maps):
    """Device-resident repeat-runner mirroring run_bass_via_pjrt's multi-core
    path, for timing NEFF executions without re-transferring inputs."""
    import jax
    from jax.experimental.shard_map import shard_map
    from jax.sharding import Mesh, NamedSharding, PartitionSpec
    from concourse import mybir
    from concourse.bass2jax import (
        _bass_exec_p,
        install_neuronx_cc_hook,
        partition_id_tensor,
    )

    install_neuronx_cc_hook()
    partition_name = nc.partition_id_tensor.name if nc.partition_id_tensor else None
    in_names, out_names, out_avals, zero_outs = [], [], [], []
    for alloc in nc.m.functions[0].allocations:
        if not isinstance(alloc, mybir.MemoryLocationSet):
            continue
        name = alloc.memorylocations[0].name
        if alloc.kind == "ExternalInput":
            if name != partition_name:
                in_names.append(name)
        elif alloc.kind == "ExternalOutput":
            out_names.append(name)
            shape = tuple(alloc.tensor_shape)
            dtype = mybir.dt.np(alloc.dtype)
            out_avals.append(jax.core.ShapedArray(shape, dtype))
            zero_outs.append(np.zeros(shape, dtype))
    n_params = len(in_names)
    n_outs = len(out_avals)
    all_in_names = list(in_names) + list(out_names)
    if partition_name is not None:
        all_in_names.append(partition_name)

    def _body(*args):
        operands = list(args)
        if partition_name is not None:
            operands.append(partition_id_tensor())
        outs = _bass_exec_p.bind(
            *operands,
            out_avals=tuple(out_avals),
            in_names=tuple(all_in_names),
            out_names=tuple(out_names),
            lowering_input_output_aliases=(),
            sim_require_finite=True,
            sim_require_nnan=True,
            nc=nc,
        )
        return tuple(outs)

    n = len(in_maps)
    devices = jax.devices()[:n]
    mesh = Mesh(np.asarray(devices), ("core",))
    in_specs = (PartitionSpec("core"),) * (n_params + n_outs)
    out_specs = (PartitionSpec("core"),) * n_outs
    donate = tuple(range(n_params, n_params + n_outs))
    sharded = jax.jit(
        shard_map(_body, mesh=mesh, in_specs=in_specs, out_specs=out_specs,
                  check_rep=False),
        donate_argnums=donate,
        keep_unused=True,
    )
    sh = NamedSharding(mesh, PartitionSpec("core"))
    dev_in = [
        jax.device_put(
            np.concatenate([np.asarray(in_maps[c][nm]) for c in range(n)], axis=0), sh
        )
        for nm in in_names
    ]

    def stage_zeros():
        return [
            jax.device_put(np.zeros((n * z.shape[0], *z.shape[1:]), z.dtype), sh)
            for z in zero_outs
        ]

    def run_batch(zero_sets):
        """Dispatch len(zero_sets) executions back-to-back, block at end."""
        outs = None
        for zeros in zero_sets:
            outs = sharded(*dev_in, *zeros)
        jax.block_until_ready(outs)
        return outs

    return run_batch, stage_zeros, out_names


def _bench_runner(make_runner_args, iters, warmup, batches):
    import time
    import jax

    run_batch, stage_zeros, _ = _make_sharded_runner(*make_runner_args)
    run_batch([stage_zeros() for _ in range(warmup)])
    raw = []
    for _ in range(batches):
        zsets = [stage_zeros() for _ in range(iters)]
        z1 = [stage_zeros()]
        jax.block_until_ready(zsets)
        jax.block_until_ready(z1)
        t0 = time.perf_counter()
        run_batch(z1)
        t1 = time.perf_counter()
        run_batch(zsets)
        t2 = time.perf_counter()
        raw.append((t1 - t0, (t2 - t1) / iters))
    return raw


def bench_device(rhs5, lhsT5_list, dataT_list, iters=20, warmup=3, batches=5):
    """Estimate per-execution device time by amortizing pipelined dispatches."""
    if "nc" not in _CACHE:
        _CACHE["nc"] = _build_nc()
    in_maps = [
        {"lhst5": lhsT5_list[c], "rhs5": rhs5, "datat": dataT_list[c]}
        for c in range(NCORES)
    ]
    times = _bench_runner((_CACHE["nc"], in_maps), iters, warmup, batches)
    if "nc_floor" not in _CACHE:
        _CACHE["nc_floor"] = _build_floor_nc()
    floor_in = [{"x_in": np.ones((128, 16), np.float32)} for _ in range(NCORES)]
    floors = _bench_runner((_CACHE["nc_floor"], floor_in), iters, warmup, batches)
    return times, floors


def _build_floor_nc():
    import concourse.tile as tile
    from concourse import bacc, mybir

    nc = bacc.Bacc("TRN2", target_bir_lowering=False)
    x_in = nc.dram_tensor("x_in", [128, 16], mybir.dt.float32, kind="ExternalInput")
    y_out = nc.dram_tensor("y_out", [128, 16], mybir.dt.float32, kind="ExternalOutput")
    with tile.TileContext(nc) as tc:
        with tc.tile_pool(name="p", bufs=1) as pool:
            t = pool.tile([128, 16], mybir.dt.float32)
            nc.sync.dma_start(out=t, in_=x_in[:, :])
            nc.sync.dma_start(out=y_out[:, :], in_=t)
    nc.compile()
    return nc


def _normal_lp_sum(x, mu, sigma, axes):
    x = np.asarray(x, np.float64)
    mu = np.asarray(mu, np.float64)
    sigma = np.asarray(sigma, np.float64)
    z = (x - mu) / sigma
    lp = -0.5 * z * z - np.log(sigma) - 0.5 * LOG_2PI
    return np.sum(lp, axis=axes)


def _reference_fallback(data, R, Weights, FactorCenters, FactorWidths,
                        MeanWeight, SigmaWeight, MeanFactorCenter,
                        SigmaFactorCenter, MeanFactorWidth, SigmaFactorWidth,
                        Snoise):
    """Pure numpy path for inputs outside the expected regime (non-constant
    Snoise). Correct for arbitrary inputs, not performance-tuned."""
    R64 = np.asarray(R, np.float64)
    C64 = np.asarray(FactorCenters, np.float64)
    w64 = np.asarray(FactorWidths, np.float64)
    lp = _normal_lp_sum(Weights, MeanWeight[None], SigmaWeight[None], (1, 2))
    lp = lp + _normal_lp_sum(FactorCenters, MeanFactorCenter[None],
                             SigmaFactorCenter[None], (1, 2))
    lp = lp + _normal_lp_sum(FactorWidths, MeanFactorWidth[None],
                             SigmaFactorWidth[None], (1,))
    data64 = np.asarray(data, np.float64)
    Sn64 = np.asarray(Snoise, np.float64)
    W64 = np.asarray(Weights, np.float64)
    r2 = np.sum(R64 * R64, axis=-1)
    c2 = np.sum(C64 * C64, axis=-1)
    CHV = 4096
    acc = np.zeros(S, np.float64)
    log_term = -np.sum(np.log(Sn64)) - 0.5 * LOG_2PI * T * V
    for v0 in range(0, V, CHV):
        v1 = min(v0 + CHV, V)
        cross = np.einsum("sfk,vk->sfv", C64, R64[v0:v1])
        d2 = r2[None, None, v0:v1] - 2.0 * cross + c2[..., None]
        Fa = np.exp(-d2 * np.exp(-w64)[..., None])
        Ym = np.einsum("stf,sfv->stv", W64, Fa)
        z = (data64[None, :, v0:v1] - Ym) / Sn64[None, :, v0:v1]
        acc += -0.5 * np.sum(z * z, axis=(1, 2))
    return (lp + acc + log_term).astype(np.float32)


def kernel(data, R, Weights, FactorCenters, FactorWidths,
           MeanWeight, SigmaWeight, MeanFactorCenter, SigmaFactorCenter,
           MeanFactorWidth, SigmaFactorWidth, Snoise, _trace=False):
    Snoise = np.asarray(Snoise)
    smin, smax = float(Snoise.min()), float(Snoise.max())
    if smin != smax or smin <= 0.0:
        return _reference_fallback(
            data, R, Weights, FactorCenters, FactorWidths, MeanWeight,
            SigmaWeight, MeanFactorCenter, SigmaFactorCenter, MeanFactorWidth,
            SigmaFactorWidth, Snoise)
    sigma = smin

    rhs5, lhsT5_list, dataT_list = _host_prep(data, R, FactorCenters, FactorWidths)
    results = _run_device(rhs5, lhsT5_list, dataT_list, trace=_trace)

    # Gather + final contractions (all O(S*T*F), fp64 on host)
    Gsum = np.zeros((T, SF), np.float64)
    Bsum = np.zeros((TH, SF), np.float64)
    Tsum = np.zeros((TH, T), np.float64)
    for r in results:
        g = np.asarray(r["g_out"], np.float64)     # [2, TH, SF]
        Gsum[:TH] += g[0]
        Gsum[TH:] += g[1]
        Bsum += np.asarray(r["b_out"], np.float64)
        Tsum += np.asarray(r["t_out"], np.float64)

    W64 = np.asarray(Weights, np.float64)          # [S, T, F]
    # t2[s] = sum_{t,f} W[s,t,f] * G[t, s*F+f]
    G3 = Gsum.reshape(T, S, F)
    t2 = np.einsum("stf,tsf->s", W64, G3)
    # B_s blocks from pair layout
    t3 = np.zeros(S, np.float64)
    for p in range(NBPAIR):
        B0 = Bsum[0:F, p * TH: p * TH + F]
        B1 = Bsum[F:2 * F, p * TH + F: p * TH + 2 * F]
        D0 = W64[2 * p].T @ W64[2 * p]
        D1 = W64[2 * p + 1].T @ W64[2 * p + 1]
        t3[2 * p] = np.sum(D0 * B0)
        t3[2 * p + 1] = np.sum(D1 * B1)
    idx = np.arange(TH)
    t1 = float(Tsum[idx, idx].sum() + Tsum[idx, TH + idx].sum())

    z2sum = (t1 - 2.0 * t2 + t3) / (sigma * sigma)
    lp_data = -0.5 * z2sum - T * V * (np.log(sigma) + 0.5 * LOG_2PI)

    lp = _normal_lp_sum(Weights, np.asarray(MeanWeight)[None],
                        np.asarray(SigmaWeight)[None], (1, 2))
    lp = lp + _normal_lp_sum(FactorCenters, np.asarray(MeanFactorCenter)[None],
                             np.asarray(SigmaFactorCenter)[None], (1, 2))
    lp = lp + _normal_lp_sum(FactorWidths, np.asarray(MeanFactorWidth)[None],
                             np.asarray(SigmaFactorWidth)[None], (1,))
    return (lp + lp_data).astype(np.float32)
